# revision 7
# baseline (speedup 1.0000x reference)
"""Bahdanau attention on 8 Trainium2 NeuronCores.

Data-parallel over batch (B=32 -> 4 per core). Weights replicated.

Per-core math (b = local batch 0..3, s in [0,2048), h/k in [0,1024)):
  proj_keys[b,s,h]  = sum_k keys[b,s,k] * Wk[h,k]
  proj_query[b,h]   = sum_k query[b,k]  * Wq[h,k]
  scores[b,s]       = sum_h tanh(proj_keys + proj_query) * We[h]
  alphas            = softmax(scores * mask)   (mask==0 -> alpha 0)
  context[b,h]      = sum_s alphas[b,s] * values[b,s,h]

All matmuls run in fp32r (TF32-like, 1 cycle/row, ~1.6e-4 rel err).
keys must have k on partitions for the PE -> transposed on-device via
PE transpose-mode (f32r, 1.5 cyc/row). Softmax uses unnormalized exp
(scores are O(1), no max subtraction needed); the denominator comes
free as a ones-column appended to the context matmul.
"""

import functools
import os
import sys

import numpy as np

sys.path.insert(0, "/opt/trn_rl_repo")

import concourse.bass as bass  # noqa: E402
import concourse.mybir as mybir  # noqa: E402
import concourse.tile as tile  # noqa: E402
from concourse import bacc, masks  # noqa: E402
from concourse.bass_utils import run_bass_kernel_spmd  # noqa: E402

F32 = mybir.dt.float32
F32R = mybir.dt.float32r

B, S, H = 32, 2048, 1024
NCORES = 8
BL = B // NCORES          # 4 batches per core
CHUNK = 512               # tokens per chunk
NCH = BL * S // CHUNK     # 16 chunks per core
KT = H // 128             # 8 k tiles
HT = H // 128             # 8 h tiles

LAST_RESULTS = None       # BassKernelResults of the most recent run (for test.py)


def _enable_ntff_tracing():
    """Inject the missing antenv.axon_hooks module so run_bass_kernel_spmd
    trace=True can capture NTFF profiles through the axon .so, and stub
    out the S3 artifact upload (zero-egress container)."""
    import types

    import antenv
    from concourse import bass_utils as _bu

    _bu.upload_artifacts = lambda tmpdir: tmpdir
    try:
        from antenv.axon_hooks import get_axon_ntff_profile_hook  # noqa: F401
        return
    except ImportError:
        pass
    if "/root/.axon_site" not in sys.path:
        sys.path.insert(0, "/root/.axon_site")
    from trn_agent_boot.trn_boot import _ntff_profile_via_ctypes

    mod = types.ModuleType("antenv.axon_hooks")
    _state = {"hook": _ntff_profile_via_ctypes("/opt/axon/libaxon_pjrt.so")}
    mod.get_axon_ntff_profile_hook = lambda: _state["hook"]
    mod.set_axon_ntff_profile_hook = lambda h: _state.update(hook=h)
    sys.modules["antenv.axon_hooks"] = mod
    antenv.axon_hooks = mod


@functools.lru_cache(maxsize=1)
def _build():
    nc = bacc.Bacc("TRN2", target_bir_lowering=False, debug=False, num_devices=NCORES)

    keys_d = nc.dram_tensor("keys", [BL * S, H], F32, kind="ExternalInput")
    vals_d = nc.dram_tensor("values", [BL * S, H], F32, kind="ExternalInput")
    wkT_d = nc.dram_tensor("wkT", [H, H], F32, kind="ExternalInput")
    wqT_d = nc.dram_tensor("wqT", [H, H], F32, kind="ExternalInput")
    qT_d = nc.dram_tensor("queryT", [128, KT * BL], F32, kind="ExternalInput")
    weR_d = nc.dram_tensor("weR", [128, 2 * HT], F32, kind="ExternalInput")
    maskC_d = nc.dram_tensor("maskC", [128, NCH * 4], F32, kind="ExternalInput")
    inv_dram = nc.dram_tensor("inv_scratch", [NCH * 4, 1], F32)
    ctx_d = nc.dram_tensor("ctx_out", [BL, H], F32, kind="ExternalOutput")
    al_d = nc.dram_tensor("alphas_out", [NCH * 4, 128], F32, kind="ExternalOutput")

    # chunked DRAM views: [chunk, p, sb, k]
    kview = keys_d.ap().rearrange("(c sb p) k -> c p sb k", sb=4, p=128)
    vview = vals_d.ap().rearrange("(c sb p) k -> c p sb k", sb=4, p=128)

    with tile.TileContext(nc) as tc:
        with (
            tc.tile_pool(name="const", bufs=1) as cpool,
            tc.tile_pool(name="wk", bufs=1) as wkpool,
            tc.tile_pool(name="wq", bufs=2) as wqpool,
            tc.tile_pool(name="knat", bufs=2) as knpool,
            tc.tile_pool(name="ktp", bufs=16) as ktpool,
            tc.tile_pool(name="vals", bufs=2) as vpool,
            tc.tile_pool(name="th", bufs=3) as thpool,
            tc.tile_pool(name="sm", bufs=2) as smpool,
            tc.tile_pool(name="ps", bufs=1, space="PSUM") as pspool,
        ):
            # ---------------- constants ----------------
            ident_f = cpool.tile([128, 128], F32, tag="identf")
            masks.make_identity(nc, ident_f[:])
            ident_r = cpool.tile([128, 128], F32R, tag="identr")
            nc.vector.tensor_copy(ident_r[:], ident_f[:])

            ones_f = cpool.tile([128, 16], F32, tag="onesf")
            nc.gpsimd.memset(ones_f[:], 1.0)
            ones_r = cpool.tile([128, 2], F32R, tag="onesr")
            nc.vector.tensor_copy(ones_r[:], ones_f[:, 0:2])

            qT_f = cpool.tile([128, KT * BL], F32, tag="qtf")
            nc.sync.dma_start(qT_f[:], qT_d.ap())
            qT_r = cpool.tile([128, KT * BL], F32R, tag="qtr")
            nc.vector.tensor_copy(qT_r[:], qT_f[:])

            weR_f = cpool.tile([128, 2 * HT], F32, tag="wef")
            nc.sync.dma_start(weR_f[:], weR_d.ap())
            weR_r = cpool.tile([128, 2 * HT], F32R, tag="wer")
            nc.vector.tensor_copy(weR_r[:], weR_f[:])

            maskC = cpool.tile([128, NCH * 4], F32, tag="maskc")
            nc.sync.dma_start(maskC[:], maskC_d.ap())

            # Wk^T tiles, rounded to f32r during DMA (SWDGE cast)
            wk_t = []
            for kt in range(KT):
                t = wkpool.tile([128, H], F32R, tag=f"wk{kt}")
                nc.gpsimd.dma_start(t[:], wkT_d.ap()[kt * 128:(kt + 1) * 128, :])
                wk_t.append(t)

            # ---------------- proj_query ----------------
            # PQ[b, h] accumulated in PSUM [4, 1024] (tag shared with ctx)
            pq_ps = pspool.tile([BL, H], F32, tag="ctx")
            for kt in range(KT):
                wq_t = wqpool.tile([128, H], F32R, tag="wq")
                nc.gpsimd.dma_start(wq_t[:], wqT_d.ap()[kt * 128:(kt + 1) * 128, :])
                for h0 in (0, 512):
                    nc.tensor.matmul(
                        pq_ps[:, h0:h0 + 512],
                        qT_r[:, kt * BL:(kt + 1) * BL],
                        wq_t[:, h0:h0 + 512],
                        start=(kt == 0), stop=(kt == KT - 1),
                    )
            pq_sb = smpool.tile([BL, H], F32, tag="pqsb", bufs=1)
            nc.vector.tensor_copy(pq_sb[:], pq_ps[:])
            # transpose to bias columns: pq_cols[:, ht*4 + b] = PQ[b, ht*128 + p]
            pq_cols = cpool.tile([128, HT * BL], F32, tag="pqcols")
            for ht in range(HT):
                ptr = pspool.tile([128, 8], F32, tag="sc")
                nc.tensor.transpose(
                    ptr[:, 0:BL], pq_sb[:, ht * 128:(ht + 1) * 128], ident_f[0:BL, 0:BL]
                )
                nc.vector.tensor_copy(pq_cols[:, ht * BL:(ht + 1) * BL], ptr[:, 0:BL])

            # ---------------- persistent accumulators ----------------
            exp_cols = cpool.tile([128, NCH * 4], F32, tag="expcols")   # masked exp(scores)
            ecol_r = cpool.tile([128, NCH * 4], F32R, tag="ecolr")    # same, rounded for PE
            ctx_ps = pspool.tile([1, H], F32, tag="ctx")       # per-batch context accumulator
            inv_row = smpool.tile([1, NCH * 4], F32, tag="invrow", bufs=1)  # 1/sum per column group

            # ---------------- main loop over chunks ----------------
            for c in range(NCH):
                b = c // 4
                cc = c % 4

                knat = knpool.tile([128, 4, H], F32R, tag="knat")
                nc.gpsimd.dma_start(knat[:], kview[c])
                vals = vpool.tile([128, 4, H], F32R, tag="vals")
                nc.gpsimd.dma_start(vals[:], vview[c])

                # transpose keys chunk: kt_tiles[kt][kp, sf] = keys[c*512+sf, kt*128+kp]
                kt_tiles = []
                for kt in range(KT):
                    trp = pspool.tile([128, CHUNK], F32R, tag="tr", bufs=2)
                    for sb in range(4):
                        nc.tensor.transpose(
                            trp[:, sb * 128:(sb + 1) * 128],
                            knat[:, sb, kt * 128:(kt + 1) * 128],
                            ident_r[:],
                        )
                    ktt = ktpool.tile([128, CHUNK], F32R, tag="kt")
                    nc.vector.tensor_copy(ktt[:], trp[:])
                    kt_tiles.append(ktt)

                # proj + tanh + scores
                # NB: each scores matmul is its own accumulation group writing a
                # distinct column pair (start=True zeroes has_written for the
                # whole 2KB PSUM bank, so interleaved groups in one bank corrupt
                # each other); the h-tile partial sums are reduced on DVE below.
                sc_ps = pspool.tile([128, 8 * HT], F32, tag="sc")
                for ht in range(HT):
                    proj = pspool.tile([128, CHUNK], F32, tag="proj", bufs=2)
                    for kt in range(KT):
                        nc.tensor.matmul(
                            proj[:],
                            wk_t[kt][:, ht * 128:(ht + 1) * 128],
                            kt_tiles[kt][:],
                            start=(kt == 0), stop=(kt == KT - 1),
                        )
                    th = thpool.tile([128, CHUNK], F32R, tag="th")
                    nc.scalar.activation(
                        th[:], proj[:], mybir.ActivationFunctionType.Tanh,
                        bias=pq_cols[:, ht * BL + b:ht * BL + b + 1],
                    )
                    for sub in range(4):
                        nc.tensor.matmul(
                            sc_ps[:, sub * 16 + ht * 2:sub * 16 + ht * 2 + 2],
                            th[:, sub * 128:(sub + 1) * 128],
                            weR_r[:, 2 * ht:2 * ht + 2],
                            start=True, stop=True,
                        )

                # reduce h-tile partials: scT[p, sub] = sum_ht sc_ps[p, sub*16 + ht*2]
                scT = smpool.tile([128, 4], F32, tag="sct")
                nc.vector.reduce_sum(
                    scT[:].rearrange("p (s o) -> p s o", o=1),
                    sc_ps[:].rearrange("p (s ht two) -> p s (ht two)", s=4, ht=8)[:, :, 0:16:2],
                    axis=mybir.AxisListType.X,
                )

                # exp + mask -> exp_cols[:, c*4 + sub]
                eraw = smpool.tile([128, 4], F32, tag="eraw")
                nc.scalar.activation(
                    eraw[:], scT[:], mybir.ActivationFunctionType.Exp
                )
                nc.vector.tensor_mul(
                    exp_cols[:, c * 4:(c + 1) * 4], eraw[:], maskC[:, c * 4:(c + 1) * 4]
                )
                nc.vector.tensor_copy(
                    ecol_r[:, c * 4:(c + 1) * 4], exp_cols[:, c * 4:(c + 1) * 4]
                )

                # per-partition partial softmax denominators for this chunk
                racc = smpool.tile([128, 1], F32, tag="racc")
                nc.vector.reduce_sum(
                    racc[:], exp_cols[:, c * 4:(c + 1) * 4], axis=mybir.AxisListType.X
                )
                if cc == 0:
                    sumacc = smpool.tile([128, 1], F32, tag="sumacc")
                    nc.vector.tensor_copy(sumacc[:], racc[:])
                else:
                    nc.vector.tensor_add(sumacc[:], sumacc[:], racc[:])

                # context + denominator accumulation (batch rows go to psum
                # partition 0; chunks are batch-contiguous so we evict per batch)
                for sub in range(4):
                    col = ecol_r[:, c * 4 + sub:c * 4 + sub + 1]
                    for h0 in (0, 512):
                        nc.tensor.matmul(
                            ctx_ps[0:1, h0:h0 + 512],
                            col,
                            vals[:, sub, h0:h0 + 512],
                            start=(cc == 0 and sub == 0),
                            stop=(cc == 3 and sub == 3),
                        )

                if cc == 3:
                    # batch b complete: cross-partition denominator via PE ones
                    sacc_r = smpool.tile([128, 1], F32R, tag="saccr")
                    nc.vector.tensor_copy(sacc_r[:], sumacc[:])
                    sums_sc = pspool.tile([1, 2], F32, tag="sc")
                    nc.tensor.matmul(sums_sc[0:1, 0:2], sacc_r[:], ones_r[:],
                                     start=True, stop=True)
                    # evict scaled context, record 1/sum
                    rcp_b = smpool.tile([1, 1], F32, tag="rcpb", bufs=4)
                    nc.vector.reciprocal(rcp_b[:], sums_sc[0:1, 0:1])
                    ctx_sb = smpool.tile([1, H], F32, tag="ctxsb", bufs=2)
                    nc.vector.tensor_scalar_mul(ctx_sb[:], ctx_ps[0:1, :], rcp_b[:])
                    nc.sync.dma_start(ctx_d.ap()[b:b + 1, :], ctx_sb[:])
                    nc.vector.tensor_scalar_mul(
                        inv_row[0:1, b * 16:(b + 1) * 16], ones_f[0:1, 0:16], rcp_b[:]
                    )

            # ---------------- epilogue: alphas ----------------
            # transpose raw exp columns to rows [64, 128]
            al_ps = pspool.tile([NCH * 4, 128], F32, tag="tr", bufs=2)
            nc.tensor.transpose(al_ps[:], exp_cols[:], ident_f[:])
            # scatter 1/sum values to one-per-partition layout [64, 1]
            # via a DRAM bounce (DRAM APs reshape freely; SBUF ones don't)
            nc.sync.dma_start(inv_dram.ap().rearrange("j one -> one j"), inv_row[0:1, :])
            inv64_sb = smpool.tile([NCH * 4, 1], F32, tag="i64", bufs=1)
            nc.sync.dma_start(inv64_sb[:], inv_dram.ap())
            al_sb = smpool.tile([NCH * 4, 128], F32, tag="alsb", bufs=1)
            nc.vector.tensor_scalar_mul(al_sb[:], al_ps[:], inv64_sb[:])
            nc.sync.dma_start(al_d.ap(), al_sb[:])

    nc.compile()
    return nc


def kernel(query, mask, values, keys, Wk, Wq, We):
    global LAST_RESULTS
    query = np.asarray(query, dtype=np.float32)
    mask = np.asarray(mask)
    values = np.asarray(values, dtype=np.float32)
    keys = np.asarray(keys, dtype=np.float32)
    Wk = np.asarray(Wk, dtype=np.float32)
    Wq = np.asarray(Wq, dtype=np.float32)
    We = np.asarray(We, dtype=np.float32)

    nc = _build()

    wkT = np.ascontiguousarray(Wk.T)                     # [k, h]
    wqT = np.ascontiguousarray(Wq.T)                     # [k, h]
    # weR[p, 2t + j] = We[0, t*128 + p]
    weR = np.ascontiguousarray(np.repeat(We[0].reshape(HT, 128).T, 2, axis=1))
    in_maps = []
    for core in range(NCORES):
        sl = slice(core * BL, (core + 1) * BL)
        q = query[sl, 0, :]                              # [4, 1024]
        # queryT[p, t*4 + b] = q[b, t*128 + p]
        qT = np.ascontiguousarray(q.reshape(BL, KT, 128).transpose(2, 1, 0).reshape(128, KT * BL))
        m = mask[sl, 0, :].astype(np.float32)            # [4, 2048]
        # maskC[p, b*16 + cc*4 + sub] = m[b, cc*512 + sub*128 + p]
        mC = np.ascontiguousarray(m.reshape(BL, 4, 4, 128).transpose(3, 0, 1, 2).reshape(128, NCH * 4))
        in_maps.append({
            "keys": np.ascontiguousarray(keys[sl].reshape(BL * S, H)),
            "values": np.ascontiguousarray(values[sl].reshape(BL * S, H)),
            "wkT": wkT,
            "wqT": wqT,
            "queryT": qT,
            "weR": weR,
            "maskC": mC,
        })

    trace = bool(os.environ.get("KERNEL_TRACE"))
    if trace:
        _enable_ntff_tracing()
    res = run_bass_kernel_spmd(nc, in_maps, core_ids=list(range(NCORES)), trace=trace)
    LAST_RESULTS = res

    context = np.empty((B, 1, H), dtype=np.float32)
    alphas = np.empty((B, 1, S), dtype=np.float32)
    for core in range(NCORES):
        r = res.results[core]
        context[core * BL:(core + 1) * BL, 0, :] = r["ctx_out"]
        alphas[core * BL:(core + 1) * BL, 0, :] = r["alphas_out"].reshape(BL, S)
    return context, alphas


# revision 8
# speedup vs baseline: 1.0992x; 1.0992x over previous
"""Bahdanau attention on 8 Trainium2 NeuronCores.

Data-parallel over batch (B=32 -> 4 per core). Weights replicated.

Per-core math (b = local batch 0..3, s in [0,2048), h/k in [0,1024)):
  proj_keys[b,s,h]  = sum_k keys[b,s,k] * Wk[h,k]
  proj_query[b,h]   = sum_k query[b,k]  * Wq[h,k]
  scores[b,s]       = sum_h tanh(proj_keys + proj_query) * We[h]
  alphas            = softmax(scores * mask)   (mask==0 -> alpha 0)
  context[b,h]      = sum_s alphas[b,s] * values[b,s,h]

All matmuls run in fp32r (TF32-like, 1 cycle/row, ~1.6e-4 rel err).
keys must have k on partitions for the PE -> transposed on-device via
PE transpose-mode (f32r, 1.5 cyc/row). Softmax uses unnormalized exp
(scores are O(1), no max subtraction needed); the denominator comes
free as a ones-column appended to the context matmul.
"""

import functools
import os
import sys

import numpy as np

sys.path.insert(0, "/opt/trn_rl_repo")

import concourse.bass as bass  # noqa: E402
import concourse.mybir as mybir  # noqa: E402
import concourse.tile as tile  # noqa: E402
from concourse import bacc, masks  # noqa: E402
from concourse.bass_utils import run_bass_kernel_spmd  # noqa: E402

F32 = mybir.dt.float32
F32R = mybir.dt.float32r

B, S, H = 32, 2048, 1024
NCORES = 8
BL = B // NCORES          # 4 batches per core
CHUNK = 512               # tokens per chunk
NCH = BL * S // CHUNK     # 16 chunks per core
KT = H // 128             # 8 k tiles
HT = H // 128             # 8 h tiles

LAST_RESULTS = None       # BassKernelResults of the most recent run (for test.py)


def _enable_ntff_tracing():
    """Inject the missing antenv.axon_hooks module so run_bass_kernel_spmd
    trace=True can capture NTFF profiles through the axon .so, and stub
    out the S3 artifact upload (zero-egress container)."""
    import types

    import antenv
    from concourse import bass_utils as _bu

    _bu.upload_artifacts = lambda tmpdir: tmpdir
    try:
        from antenv.axon_hooks import get_axon_ntff_profile_hook  # noqa: F401
        return
    except ImportError:
        pass
    if "/root/.axon_site" not in sys.path:
        sys.path.insert(0, "/root/.axon_site")
    from trn_agent_boot.trn_boot import _ntff_profile_via_ctypes

    mod = types.ModuleType("antenv.axon_hooks")
    _state = {"hook": _ntff_profile_via_ctypes("/opt/axon/libaxon_pjrt.so")}
    mod.get_axon_ntff_profile_hook = lambda: _state["hook"]
    mod.set_axon_ntff_profile_hook = lambda h: _state.update(hook=h)
    sys.modules["antenv.axon_hooks"] = mod
    antenv.axon_hooks = mod


@functools.lru_cache(maxsize=1)
def _build():
    nc = bacc.Bacc("TRN2", target_bir_lowering=False, debug=False, num_devices=NCORES)

    keys_d = nc.dram_tensor("keys", [BL * S, H], F32, kind="ExternalInput")
    vals_d = nc.dram_tensor("values", [BL * S, H], F32, kind="ExternalInput")
    wkT_d = nc.dram_tensor("wkT", [H, H], F32, kind="ExternalInput")
    wqT_d = nc.dram_tensor("wqT", [H, H], F32, kind="ExternalInput")
    qT_d = nc.dram_tensor("queryT", [128, KT * BL], F32, kind="ExternalInput")
    weR_d = nc.dram_tensor("weR", [128, 2 * HT], F32, kind="ExternalInput")
    maskC_d = nc.dram_tensor("maskC", [128, NCH * 4], F32, kind="ExternalInput")
    inv_dram = nc.dram_tensor("inv_scratch", [NCH * 4, 1], F32)
    ctx_d = nc.dram_tensor("ctx_out", [BL, H], F32, kind="ExternalOutput")
    al_d = nc.dram_tensor("alphas_out", [NCH * 4, 128], F32, kind="ExternalOutput")

    # chunked DRAM views: [chunk, p, sb, k]
    kview = keys_d.ap().rearrange("(c sb p) k -> c p sb k", sb=4, p=128)
    vview = vals_d.ap().rearrange("(c sb p) k -> c p sb k", sb=4, p=128)

    with tile.TileContext(nc) as tc:
        with (
            tc.tile_pool(name="const", bufs=1) as cpool,
            tc.tile_pool(name="wk", bufs=1) as wkpool,
            tc.tile_pool(name="wq", bufs=2) as wqpool,
            tc.tile_pool(name="knat", bufs=3) as knpool,
            tc.tile_pool(name="ktp", bufs=16) as ktpool,
            tc.tile_pool(name="vals", bufs=3) as vpool,
            tc.tile_pool(name="th", bufs=3) as thpool,
            tc.tile_pool(name="sm", bufs=2) as smpool,
            tc.tile_pool(name="ps", bufs=1, space="PSUM") as pspool,
        ):
            # ---------------- constants ----------------
            ident_f = cpool.tile([128, 128], F32, tag="identf")
            masks.make_identity(nc, ident_f[:])
            ident_r = cpool.tile([128, 128], F32R, tag="identr")
            nc.vector.tensor_copy(ident_r[:], ident_f[:])

            ones_f = cpool.tile([128, 16], F32, tag="onesf")
            nc.gpsimd.memset(ones_f[:], 1.0)
            ones_r = cpool.tile([128, 2], F32R, tag="onesr")
            nc.vector.tensor_copy(ones_r[:], ones_f[:, 0:2])

            qT_f = cpool.tile([128, KT * BL], F32, tag="qtf")
            nc.sync.dma_start(qT_f[:], qT_d.ap())
            qT_r = cpool.tile([128, KT * BL], F32R, tag="qtr")
            nc.vector.tensor_copy(qT_r[:], qT_f[:])

            weR_f = cpool.tile([128, 2 * HT], F32, tag="wef")
            nc.sync.dma_start(weR_f[:], weR_d.ap())
            weR_r = cpool.tile([128, 2 * HT], F32R, tag="wer")
            nc.vector.tensor_copy(weR_r[:], weR_f[:])

            maskC = cpool.tile([128, NCH * 4], F32, tag="maskc")
            nc.sync.dma_start(maskC[:], maskC_d.ap())

            # Wk^T tiles, rounded to f32r during DMA (SWDGE cast)
            wk_t = []
            for kt in range(KT):
                t = wkpool.tile([128, H], F32R, tag=f"wk{kt}")
                nc.gpsimd.dma_start(t[:], wkT_d.ap()[kt * 128:(kt + 1) * 128, :])
                wk_t.append(t)

            # ---------------- proj_query ----------------
            # PQ[b, h] accumulated in PSUM [4, 1024] (tag shared with ctx)
            pq_ps = pspool.tile([BL, H], F32, tag="ctx")
            for kt in range(KT):
                wq_t = wqpool.tile([128, H], F32R, tag="wq")
                nc.gpsimd.dma_start(wq_t[:], wqT_d.ap()[kt * 128:(kt + 1) * 128, :])
                for h0 in (0, 512):
                    nc.tensor.matmul(
                        pq_ps[:, h0:h0 + 512],
                        qT_r[:, kt * BL:(kt + 1) * BL],
                        wq_t[:, h0:h0 + 512],
                        start=(kt == 0), stop=(kt == KT - 1),
                    )
            pq_sb = smpool.tile([BL, H], F32, tag="pqsb", bufs=1)
            nc.vector.tensor_copy(pq_sb[:], pq_ps[:])
            # transpose to bias columns: pq_cols[:, ht*4 + b] = PQ[b, ht*128 + p]
            pq_cols = cpool.tile([128, HT * BL], F32, tag="pqcols")
            for ht in range(HT):
                ptr = pspool.tile([128, 8], F32, tag="sc", bufs=2)
                nc.tensor.transpose(
                    ptr[:, 0:BL], pq_sb[:, ht * 128:(ht + 1) * 128], ident_f[0:BL, 0:BL]
                )
                nc.vector.tensor_copy(pq_cols[:, ht * BL:(ht + 1) * BL], ptr[:, 0:BL])

            # ---------------- persistent accumulators ----------------
            exp_cols = cpool.tile([128, NCH * 4], F32, tag="expcols")   # masked exp(scores)
            ecol_r = cpool.tile([128, NCH * 4], F32R, tag="ecolr")    # same, rounded for PE
            ctx_ps = pspool.tile([1, H], F32, tag="ctx")       # per-batch context accumulator
            inv_row = smpool.tile([1, NCH * 4], F32, tag="invrow", bufs=1)  # 1/sum per column group

            # ---------------- main loop over chunks ----------------
            for c in range(NCH):
                b = c // 4
                cc = c % 4

                knat = knpool.tile([128, 4, H], F32R, tag="knat")
                nc.gpsimd.dma_start(knat[:], kview[c])
                vals = vpool.tile([128, 4, H], F32R, tag="vals")
                nc.gpsimd.dma_start(vals[:], vview[c])

                # transpose keys chunk: kt_tiles[kt][kp, sf] = keys[c*512+sf, kt*128+kp]
                kt_tiles = []
                for kt in range(KT):
                    trp = pspool.tile([128, CHUNK], F32R, tag="tr", bufs=2)
                    for sb in range(4):
                        nc.tensor.transpose(
                            trp[:, sb * 128:(sb + 1) * 128],
                            knat[:, sb, kt * 128:(kt + 1) * 128],
                            ident_r[:],
                        )
                    ktt = ktpool.tile([128, CHUNK], F32R, tag="kt")
                    nc.vector.tensor_copy(ktt[:], trp[:])
                    kt_tiles.append(ktt)

                # proj + tanh + scores.
                # Scores: We column is the stationary operand (1-column weight
                # load is ~free), tanh tiles stream as the N=512 moving operand;
                # the row accumulates over h tiles in one PSUM group.
                scrow = pspool.tile([1, CHUNK], F32, tag="sc", bufs=2)
                for ht in range(HT):
                    proj = pspool.tile([128, CHUNK], F32, tag="proj", bufs=2)
                    for kt in range(KT):
                        nc.tensor.matmul(
                            proj[:],
                            wk_t[kt][:, ht * 128:(ht + 1) * 128],
                            kt_tiles[kt][:],
                            start=(kt == 0), stop=(kt == KT - 1),
                        )
                    th = thpool.tile([128, CHUNK], F32R, tag="th")
                    nc.scalar.activation(
                        th[:], proj[:], mybir.ActivationFunctionType.Tanh,
                        bias=pq_cols[:, ht * BL + b:ht * BL + b + 1],
                    )
                    nc.tensor.matmul(
                        scrow[0:1, :],
                        weR_r[:, 2 * ht:2 * ht + 1],
                        th[:],
                        start=(ht == 0), stop=(ht == HT - 1),
                    )

                # row -> columns: SBUF bounce + 4 single-column PE transposes
                # (independent groups into distinct columns of one bank)
                scrow_sb = smpool.tile([1, CHUNK], F32, tag="scrow")
                nc.scalar.copy(scrow_sb[:], scrow[0:1, :])
                sct_ps = pspool.tile([128, 4], F32, tag="sc", bufs=2)
                for sub in range(4):
                    nc.tensor.transpose(
                        sct_ps[:, sub:sub + 1],
                        scrow_sb[0:1, sub * 128:(sub + 1) * 128],
                        ident_f[0:1, 0:1],
                    )
                scT = smpool.tile([128, 4], F32, tag="sct")
                nc.vector.tensor_copy(scT[:], sct_ps[:])

                # exp + mask -> exp_cols[:, c*4 + sub]
                eraw = smpool.tile([128, 4], F32, tag="eraw")
                nc.scalar.activation(
                    eraw[:], scT[:], mybir.ActivationFunctionType.Exp
                )
                nc.vector.tensor_mul(
                    exp_cols[:, c * 4:(c + 1) * 4], eraw[:], maskC[:, c * 4:(c + 1) * 4]
                )
                nc.vector.tensor_copy(
                    ecol_r[:, c * 4:(c + 1) * 4], exp_cols[:, c * 4:(c + 1) * 4]
                )

                # per-partition partial softmax denominators for this chunk
                racc = smpool.tile([128, 1], F32, tag="racc")
                nc.vector.reduce_sum(
                    racc[:], exp_cols[:, c * 4:(c + 1) * 4], axis=mybir.AxisListType.X
                )
                if cc == 0:
                    sumacc = smpool.tile([128, 1], F32, tag="sumacc")
                    nc.vector.tensor_copy(sumacc[:], racc[:])
                else:
                    nc.vector.tensor_add(sumacc[:], sumacc[:], racc[:])

                # context + denominator accumulation (batch rows go to psum
                # partition 0; chunks are batch-contiguous so we evict per batch)
                for sub in range(4):
                    col = ecol_r[:, c * 4 + sub:c * 4 + sub + 1]
                    for h0 in (0, 512):
                        nc.tensor.matmul(
                            ctx_ps[0:1, h0:h0 + 512],
                            col,
                            vals[:, sub, h0:h0 + 512],
                            start=(cc == 0 and sub == 0),
                            stop=(cc == 3 and sub == 3),
                        )

                if cc == 3:
                    # batch b complete: cross-partition denominator via PE ones
                    sacc_r = smpool.tile([128, 1], F32R, tag="saccr")
                    nc.vector.tensor_copy(sacc_r[:], sumacc[:])
                    sums_sc = pspool.tile([1, 2], F32, tag="sc", bufs=2)
                    nc.tensor.matmul(sums_sc[0:1, 0:2], sacc_r[:], ones_r[:],
                                     start=True, stop=True)
                    # evict scaled context, record 1/sum
                    rcp_b = smpool.tile([1, 1], F32, tag="rcpb", bufs=4)
                    nc.vector.reciprocal(rcp_b[:], sums_sc[0:1, 0:1])
                    ctx_sb = smpool.tile([1, H], F32, tag="ctxsb", bufs=2)
                    nc.vector.tensor_scalar_mul(ctx_sb[:], ctx_ps[0:1, :], rcp_b[:])
                    nc.sync.dma_start(ctx_d.ap()[b:b + 1, :], ctx_sb[:])
                    nc.vector.tensor_scalar_mul(
                        inv_row[0:1, b * 16:(b + 1) * 16], ones_f[0:1, 0:16], rcp_b[:]
                    )

            # ---------------- epilogue: alphas ----------------
            # transpose raw exp columns to rows [64, 128]
            al_ps = pspool.tile([NCH * 4, 128], F32, tag="tr", bufs=2)
            nc.tensor.transpose(al_ps[:], exp_cols[:], ident_f[:])
            # scatter 1/sum values to one-per-partition layout [64, 1]
            # via a DRAM bounce (DRAM APs reshape freely; SBUF ones don't)
            nc.sync.dma_start(inv_dram.ap().rearrange("j one -> one j"), inv_row[0:1, :])
            inv64_sb = smpool.tile([NCH * 4, 1], F32, tag="i64", bufs=1)
            nc.sync.dma_start(inv64_sb[:], inv_dram.ap())
            al_sb = smpool.tile([NCH * 4, 128], F32, tag="alsb", bufs=1)
            nc.vector.tensor_scalar_mul(al_sb[:], al_ps[:], inv64_sb[:])
            nc.sync.dma_start(al_d.ap(), al_sb[:])

    nc.compile()
    return nc


def kernel(query, mask, values, keys, Wk, Wq, We):
    global LAST_RESULTS
    query = np.asarray(query, dtype=np.float32)
    mask = np.asarray(mask)
    values = np.asarray(values, dtype=np.float32)
    keys = np.asarray(keys, dtype=np.float32)
    Wk = np.asarray(Wk, dtype=np.float32)
    Wq = np.asarray(Wq, dtype=np.float32)
    We = np.asarray(We, dtype=np.float32)

    nc = _build()

    wkT = np.ascontiguousarray(Wk.T)                     # [k, h]
    wqT = np.ascontiguousarray(Wq.T)                     # [k, h]
    # weR[p, 2t + j] = We[0, t*128 + p]
    weR = np.ascontiguousarray(np.repeat(We[0].reshape(HT, 128).T, 2, axis=1))
    in_maps = []
    for core in range(NCORES):
        sl = slice(core * BL, (core + 1) * BL)
        q = query[sl, 0, :]                              # [4, 1024]
        # queryT[p, t*4 + b] = q[b, t*128 + p]
        qT = np.ascontiguousarray(q.reshape(BL, KT, 128).transpose(2, 1, 0).reshape(128, KT * BL))
        m = mask[sl, 0, :].astype(np.float32)            # [4, 2048]
        # maskC[p, b*16 + cc*4 + sub] = m[b, cc*512 + sub*128 + p]
        mC = np.ascontiguousarray(m.reshape(BL, 4, 4, 128).transpose(3, 0, 1, 2).reshape(128, NCH * 4))
        in_maps.append({
            "keys": np.ascontiguousarray(keys[sl].reshape(BL * S, H)),
            "values": np.ascontiguousarray(values[sl].reshape(BL * S, H)),
            "wkT": wkT,
            "wqT": wqT,
            "queryT": qT,
            "weR": weR,
            "maskC": mC,
        })

    trace = bool(os.environ.get("KERNEL_TRACE"))
    if trace:
        _enable_ntff_tracing()
    res = run_bass_kernel_spmd(nc, in_maps, core_ids=list(range(NCORES)), trace=trace)
    LAST_RESULTS = res

    context = np.empty((B, 1, H), dtype=np.float32)
    alphas = np.empty((B, 1, S), dtype=np.float32)
    for core in range(NCORES):
        r = res.results[core]
        context[core * BL:(core + 1) * BL, 0, :] = r["ctx_out"]
        alphas[core * BL:(core + 1) * BL, 0, :] = r["alphas_out"].reshape(BL, S)
    return context, alphas


# revision 9
# speedup vs baseline: 1.1095x; 1.0093x over previous
"""Bahdanau attention on 8 Trainium2 NeuronCores.

Data-parallel over batch (B=32 -> 4 per core). Weights replicated.

Per-core math (b = local batch 0..3, s in [0,2048), h/k in [0,1024)):
  proj_keys[b,s,h]  = sum_k keys[b,s,k] * Wk[h,k]
  proj_query[b,h]   = sum_k query[b,k]  * Wq[h,k]
  scores[b,s]       = sum_h tanh(proj_keys + proj_query) * We[h]
  alphas            = softmax(scores * mask)   (mask==0 -> alpha 0)
  context[b,h]      = sum_s alphas[b,s] * values[b,s,h]

All matmuls run in fp32r (TF32-like, 1 cycle/row, ~1.6e-4 rel err).
keys must have k on partitions for the PE -> transposed on-device via
PE transpose-mode (f32r, 1.5 cyc/row). Softmax uses unnormalized exp
(scores are O(1), no max subtraction needed); the denominator comes
free as a ones-column appended to the context matmul.
"""

import functools
import os
import sys

import numpy as np

sys.path.insert(0, "/opt/trn_rl_repo")

import concourse.bass as bass  # noqa: E402
import concourse.mybir as mybir  # noqa: E402
import concourse.tile as tile  # noqa: E402
from concourse import bacc, masks  # noqa: E402
from concourse.bass_utils import run_bass_kernel_spmd  # noqa: E402

F32 = mybir.dt.float32
F32R = mybir.dt.float32r

B, S, H = 32, 2048, 1024
NCORES = 8
BL = B // NCORES          # 4 batches per core
CHUNK = 512               # tokens per chunk
NCH = BL * S // CHUNK     # 16 chunks per core
KT = H // 128             # 8 k tiles
HT = H // 128             # 8 h tiles

LAST_RESULTS = None       # BassKernelResults of the most recent run (for test.py)


def _enable_ntff_tracing():
    """Inject the missing antenv.axon_hooks module so run_bass_kernel_spmd
    trace=True can capture NTFF profiles through the axon .so, and stub
    out the S3 artifact upload (zero-egress container)."""
    import types

    import antenv
    from concourse import bass_utils as _bu

    _bu.upload_artifacts = lambda tmpdir: tmpdir
    try:
        from antenv.axon_hooks import get_axon_ntff_profile_hook  # noqa: F401
        return
    except ImportError:
        pass
    if "/root/.axon_site" not in sys.path:
        sys.path.insert(0, "/root/.axon_site")
    from trn_agent_boot.trn_boot import _ntff_profile_via_ctypes

    mod = types.ModuleType("antenv.axon_hooks")
    _state = {"hook": _ntff_profile_via_ctypes("/opt/axon/libaxon_pjrt.so")}
    mod.get_axon_ntff_profile_hook = lambda: _state["hook"]
    mod.set_axon_ntff_profile_hook = lambda h: _state.update(hook=h)
    sys.modules["antenv.axon_hooks"] = mod
    antenv.axon_hooks = mod


@functools.lru_cache(maxsize=1)
def _build():
    nc = bacc.Bacc("TRN2", target_bir_lowering=False, debug=False, num_devices=NCORES)

    keys_d = nc.dram_tensor("keys", [BL * S, H], F32, kind="ExternalInput")
    vals_d = nc.dram_tensor("values", [BL * S, H], F32, kind="ExternalInput")
    wkT_d = nc.dram_tensor("wkT", [H, H], F32, kind="ExternalInput")
    wqT_d = nc.dram_tensor("wqT", [H, H], F32, kind="ExternalInput")
    qT_d = nc.dram_tensor("queryT", [128, KT * BL], F32, kind="ExternalInput")
    weR_d = nc.dram_tensor("weR", [128, 2 * HT], F32, kind="ExternalInput")
    maskC_d = nc.dram_tensor("maskC", [128, NCH * 4], F32, kind="ExternalInput")
    inv_dram = nc.dram_tensor("inv_scratch", [NCH * 4, 1], F32)
    ctx_d = nc.dram_tensor("ctx_out", [BL, H], F32, kind="ExternalOutput")
    al_d = nc.dram_tensor("alphas_out", [NCH * 4, 128], F32, kind="ExternalOutput")

    # chunked DRAM views: [chunk, p, sb, k]
    kview = keys_d.ap().rearrange("(c sb p) k -> c p sb k", sb=4, p=128)
    vview = vals_d.ap().rearrange("(c sb p) k -> c p sb k", sb=4, p=128)

    with tile.TileContext(nc) as tc:
        with (
            tc.tile_pool(name="const", bufs=1) as cpool,
            tc.tile_pool(name="wk", bufs=1) as wkpool,
            tc.tile_pool(name="wq", bufs=2) as wqpool,
            tc.tile_pool(name="knat", bufs=3) as knpool,
            tc.tile_pool(name="ktp", bufs=16) as ktpool,
            tc.tile_pool(name="vals", bufs=3) as vpool,
            tc.tile_pool(name="th", bufs=3) as thpool,
            tc.tile_pool(name="sm", bufs=2) as smpool,
            tc.tile_pool(name="ps", bufs=1, space="PSUM") as pspool,
        ):
            # ---------------- constants ----------------
            ident_f = cpool.tile([128, 128], F32, tag="identf")
            masks.make_identity(nc, ident_f[:])
            ident_r = cpool.tile([128, 128], F32R, tag="identr")
            nc.vector.tensor_copy(ident_r[:], ident_f[:])

            ones_f = cpool.tile([128, 16], F32, tag="onesf")
            nc.gpsimd.memset(ones_f[:], 1.0)
            ones_r = cpool.tile([128, 2], F32R, tag="onesr")
            nc.vector.tensor_copy(ones_r[:], ones_f[:, 0:2])

            qT_f = cpool.tile([128, KT * BL], F32, tag="qtf")
            nc.sync.dma_start(qT_f[:], qT_d.ap())
            qT_r = cpool.tile([128, KT * BL], F32R, tag="qtr")
            nc.vector.tensor_copy(qT_r[:], qT_f[:])

            weR_f = cpool.tile([128, 2 * HT], F32, tag="wef")
            nc.sync.dma_start(weR_f[:], weR_d.ap())
            weR_r = cpool.tile([128, 2 * HT], F32R, tag="wer")
            nc.vector.tensor_copy(weR_r[:], weR_f[:])

            maskC = cpool.tile([128, NCH * 4], F32, tag="maskc")
            nc.sync.dma_start(maskC[:], maskC_d.ap())

            # Wk^T tiles, rounded to f32r during DMA (SWDGE cast)
            wk_t = []
            for kt in range(KT):
                t = wkpool.tile([128, H], F32R, tag=f"wk{kt}")
                nc.gpsimd.dma_start(t[:], wkT_d.ap()[kt * 128:(kt + 1) * 128, :])
                wk_t.append(t)

            # ---------------- proj_query ----------------
            # PQ[b, h] accumulated in PSUM [4, 1024] (tag shared with ctx)
            pq_ps = pspool.tile([BL, H], F32, tag="ctx")
            for kt in range(KT):
                wq_t = wqpool.tile([128, H], F32R, tag="wq")
                nc.gpsimd.dma_start(wq_t[:], wqT_d.ap()[kt * 128:(kt + 1) * 128, :])
                for h0 in (0, 512):
                    nc.tensor.matmul(
                        pq_ps[:, h0:h0 + 512],
                        qT_r[:, kt * BL:(kt + 1) * BL],
                        wq_t[:, h0:h0 + 512],
                        start=(kt == 0), stop=(kt == KT - 1),
                    )
            pq_sb = smpool.tile([BL, H], F32, tag="pqsb", bufs=1)
            nc.vector.tensor_copy(pq_sb[:], pq_ps[:])
            # transpose to bias columns: pq_cols[:, ht*4 + b] = PQ[b, ht*128 + p]
            pq_cols = cpool.tile([128, HT * BL], F32, tag="pqcols")
            for ht in range(HT):
                ptr = pspool.tile([128, 8], F32, tag="sc", bufs=2)
                nc.tensor.transpose(
                    ptr[:, 0:BL], pq_sb[:, ht * 128:(ht + 1) * 128], ident_f[0:BL, 0:BL]
                )
                nc.vector.tensor_copy(pq_cols[:, ht * BL:(ht + 1) * BL], ptr[:, 0:BL])

            # ---------------- persistent accumulators ----------------
            exp_cols = cpool.tile([128, NCH * 4], F32, tag="expcols")   # masked exp(scores)
            ecol_r = cpool.tile([128, NCH * 4], F32R, tag="ecolr")    # same, rounded for PE
            ctx_ps = pspool.tile([1, H], F32, tag="ctx")       # per-batch context accumulator
            inv_row = smpool.tile([1, NCH * 4], F32, tag="invrow", bufs=1)  # 1/sum per column group

            # ---------------- main loop over chunks (software-pipelined) ----
            # The PE sequencer is in-order, so every PE op that depends on an
            # ACT/DVE chain is emitted with at least one independent proj
            # group queued in front of it. Chunk c's scores tail / exp chain /
            # context matmuls are deferred into chunk c+1's emission.
            knat_t = {}
            vals_t = {}

            def issue_dma(c):
                knat = knpool.tile([128, 4, H], F32R, tag="knat", name=f"knat{c}")
                nc.gpsimd.dma_start(knat[:], kview[c])
                vals = vpool.tile([128, 4, H], F32R, tag="vals", name=f"vals{c}")
                nc.gpsimd.dma_start(vals[:], vview[c])
                knat_t[c] = knat
                vals_t[c] = vals

            issue_dma(0)
            issue_dma(1)

            state = {"sumacc": None}
            carry = None  # deferred tail of the previous chunk

            def transpose_keys(c):
                knat = knat_t.pop(c)
                kt_tiles = []
                for kt in range(KT):
                    trp = pspool.tile([128, CHUNK], F32R, tag="tr", bufs=2,
                                      name=f"trp{c}_{kt}")
                    for sb in range(4):
                        nc.tensor.transpose(
                            trp[:, sb * 128:(sb + 1) * 128],
                            knat[:, sb, kt * 128:(kt + 1) * 128],
                            ident_r[:],
                        )
                    ktt = ktpool.tile([128, CHUNK], F32R, tag="kt", name=f"kt{c}_{kt}")
                    nc.vector.tensor_copy(ktt[:], trp[:])
                    kt_tiles.append(ktt)
                return kt_tiles

            def proj_group(c, ht, kt_tiles, b):
                proj = pspool.tile([128, CHUNK], F32, tag="proj", bufs=2,
                                   name=f"proj{c}_{ht}")
                for kt in range(KT):
                    nc.tensor.matmul(
                        proj[:],
                        wk_t[kt][:, ht * 128:(ht + 1) * 128],
                        kt_tiles[kt][:],
                        start=(kt == 0), stop=(kt == KT - 1),
                    )
                th = thpool.tile([128, CHUNK], F32R, tag="th", name=f"th{c}_{ht}")
                nc.scalar.activation(
                    th[:], proj[:], mybir.ActivationFunctionType.Tanh,
                    bias=pq_cols[:, ht * BL + b:ht * BL + b + 1],
                )
                return th

            def scores_mm(scrow, weht, th):
                nc.tensor.matmul(
                    scrow[0:1, :],
                    weR_r[:, 2 * weht:2 * weht + 1],
                    th[:],
                    start=(weht == 0), stop=(weht == HT - 1),
                )

            def finish_part1(f):
                # scores row -> columns -> masked exp -> rounded lhsT columns
                c = f["c"]
                sct_ps = pspool.tile([128, 4], F32, tag="sc", bufs=2,
                                     name=f"sct{c}")
                for sub in range(4):
                    nc.tensor.transpose(
                        sct_ps[:, sub:sub + 1],
                        f["scrow_sb"][0:1, sub * 128:(sub + 1) * 128],
                        ident_f[0:1, 0:1],
                    )
                scT = smpool.tile([128, 4], F32, tag="sct", name=f"scT{c}")
                nc.vector.tensor_copy(scT[:], sct_ps[:])
                eraw = smpool.tile([128, 4], F32, tag="eraw", name=f"eraw{c}")
                nc.scalar.activation(eraw[:], scT[:], mybir.ActivationFunctionType.Exp)
                nc.vector.tensor_mul(
                    exp_cols[:, c * 4:(c + 1) * 4], eraw[:], maskC[:, c * 4:(c + 1) * 4]
                )
                nc.vector.tensor_copy(
                    ecol_r[:, c * 4:(c + 1) * 4], exp_cols[:, c * 4:(c + 1) * 4]
                )
                racc = smpool.tile([128, 1], F32, tag="racc", name=f"racc{c}")
                nc.vector.reduce_sum(
                    racc[:], exp_cols[:, c * 4:(c + 1) * 4], axis=mybir.AxisListType.X
                )
                if f["cc"] == 0:
                    state["sumacc"] = smpool.tile([128, 1], F32, tag="sumacc",
                                                  name=f"sumacc{c}")
                    nc.vector.tensor_copy(state["sumacc"][:], racc[:])
                else:
                    nc.vector.tensor_add(state["sumacc"][:], state["sumacc"][:], racc[:])

            def finish_part2(f):
                # context accumulation; per-batch normalization + eviction
                c, cc, b, vals = f["c"], f["cc"], f["b"], f["vals"]
                for sub in range(4):
                    col = ecol_r[:, c * 4 + sub:c * 4 + sub + 1]
                    for h0 in (0, 512):
                        nc.tensor.matmul(
                            ctx_ps[0:1, h0:h0 + 512],
                            col,
                            vals[:, sub, h0:h0 + 512],
                            start=(cc == 0 and sub == 0),
                            stop=(cc == 3 and sub == 3),
                        )
                if cc == 3:
                    sacc_r = smpool.tile([128, 1], F32R, tag="saccr", name=f"saccr{b}")
                    nc.vector.tensor_copy(sacc_r[:], state["sumacc"][:])
                    sums_sc = pspool.tile([1, 2], F32, tag="sc", bufs=2,
                                          name=f"sums{b}")
                    nc.tensor.matmul(sums_sc[0:1, 0:2], sacc_r[:], ones_r[:],
                                     start=True, stop=True)
                    rcp_b = smpool.tile([1, 1], F32, tag="rcpb", bufs=4,
                                        name=f"rcp{b}")
                    nc.vector.reciprocal(rcp_b[:], sums_sc[0:1, 0:1])
                    ctx_sb = smpool.tile([1, H], F32, tag="ctxsb", bufs=2,
                                         name=f"ctxsb{b}")
                    nc.vector.tensor_scalar_mul(ctx_sb[:], ctx_ps[0:1, :], rcp_b[:])
                    nc.sync.dma_start(ctx_d.ap()[b:b + 1, :], ctx_sb[:])
                    nc.vector.tensor_scalar_mul(
                        inv_row[0:1, b * 16:(b + 1) * 16], ones_f[0:1, 0:16], rcp_b[:]
                    )

            for c in range(NCH):
                b = c // 4
                cc = c % 4
                if c + 2 < NCH:
                    issue_dma(c + 2)

                kt_tiles = transpose_keys(c)

                if carry is not None:
                    # finish the previous chunk's scores row + evict it to SBUF
                    scores_mm(carry["scrow"], HT - 1, carry["th7"])
                    carry["scrow_sb"] = smpool.tile([1, CHUNK], F32, tag="scrow",
                                                    name=f"scrow_sb{carry['c']}")
                    nc.scalar.copy(carry["scrow_sb"][:], carry["scrow"][0:1, :])

                th_prev = proj_group(c, 0, kt_tiles, b)
                if carry is not None:
                    finish_part1(carry)

                th = proj_group(c, 1, kt_tiles, b)
                if carry is not None:
                    finish_part2(carry)
                    carry = None

                scrow = pspool.tile([1, CHUNK], F32, tag="sc", bufs=2,
                                    name=f"scrow{c}")
                scores_mm(scrow, 0, th_prev)
                th_prev = th

                for ht in range(2, HT):
                    th = proj_group(c, ht, kt_tiles, b)
                    scores_mm(scrow, ht - 1, th_prev)
                    th_prev = th

                carry = {"c": c, "b": b, "cc": cc, "scrow": scrow,
                         "th7": th_prev, "vals": vals_t.pop(c)}

            # flush the last chunk
            scores_mm(carry["scrow"], HT - 1, carry["th7"])
            carry["scrow_sb"] = smpool.tile([1, CHUNK], F32, tag="scrow",
                                            name="scrow_sb_last")
            nc.scalar.copy(carry["scrow_sb"][:], carry["scrow"][0:1, :])
            finish_part1(carry)
            finish_part2(carry)
            carry = None

            # ---------------- epilogue: alphas ----------------
            # transpose raw exp columns to rows [64, 128]
            al_ps = pspool.tile([NCH * 4, 128], F32, tag="tr", bufs=2)
            nc.tensor.transpose(al_ps[:], exp_cols[:], ident_f[:])
            # scatter 1/sum values to one-per-partition layout [64, 1]
            # via a DRAM bounce (DRAM APs reshape freely; SBUF ones don't)
            nc.sync.dma_start(inv_dram.ap().rearrange("j one -> one j"), inv_row[0:1, :])
            inv64_sb = smpool.tile([NCH * 4, 1], F32, tag="i64", bufs=1)
            nc.sync.dma_start(inv64_sb[:], inv_dram.ap())
            al_sb = smpool.tile([NCH * 4, 128], F32, tag="alsb", bufs=1)
            nc.vector.tensor_scalar_mul(al_sb[:], al_ps[:], inv64_sb[:])
            nc.sync.dma_start(al_d.ap(), al_sb[:])

    nc.compile()
    return nc


def kernel(query, mask, values, keys, Wk, Wq, We):
    global LAST_RESULTS
    query = np.asarray(query, dtype=np.float32)
    mask = np.asarray(mask)
    values = np.asarray(values, dtype=np.float32)
    keys = np.asarray(keys, dtype=np.float32)
    Wk = np.asarray(Wk, dtype=np.float32)
    Wq = np.asarray(Wq, dtype=np.float32)
    We = np.asarray(We, dtype=np.float32)

    nc = _build()

    wkT = np.ascontiguousarray(Wk.T)                     # [k, h]
    wqT = np.ascontiguousarray(Wq.T)                     # [k, h]
    # weR[p, 2t + j] = We[0, t*128 + p]
    weR = np.ascontiguousarray(np.repeat(We[0].reshape(HT, 128).T, 2, axis=1))
    in_maps = []
    for core in range(NCORES):
        sl = slice(core * BL, (core + 1) * BL)
        q = query[sl, 0, :]                              # [4, 1024]
        # queryT[p, t*4 + b] = q[b, t*128 + p]
        qT = np.ascontiguousarray(q.reshape(BL, KT, 128).transpose(2, 1, 0).reshape(128, KT * BL))
        m = mask[sl, 0, :].astype(np.float32)            # [4, 2048]
        # maskC[p, b*16 + cc*4 + sub] = m[b, cc*512 + sub*128 + p]
        mC = np.ascontiguousarray(m.reshape(BL, 4, 4, 128).transpose(3, 0, 1, 2).reshape(128, NCH * 4))
        in_maps.append({
            "keys": np.ascontiguousarray(keys[sl].reshape(BL * S, H)),
            "values": np.ascontiguousarray(values[sl].reshape(BL * S, H)),
            "wkT": wkT,
            "wqT": wqT,
            "queryT": qT,
            "weR": weR,
            "maskC": mC,
        })

    trace = bool(os.environ.get("KERNEL_TRACE"))
    if trace:
        _enable_ntff_tracing()
    res = run_bass_kernel_spmd(nc, in_maps, core_ids=list(range(NCORES)), trace=trace)
    LAST_RESULTS = res

    context = np.empty((B, 1, H), dtype=np.float32)
    alphas = np.empty((B, 1, S), dtype=np.float32)
    for core in range(NCORES):
        r = res.results[core]
        context[core * BL:(core + 1) * BL, 0, :] = r["ctx_out"]
        alphas[core * BL:(core + 1) * BL, 0, :] = r["alphas_out"].reshape(BL, S)
    return context, alphas
